# revision 36
# baseline (speedup 1.0000x reference)
"""Trainium2 Bass kernel for nn_LocationAwareMSAGAT_Net.

Data-parallel over batch B=8 across 8 NeuronCores (one batch element per
core); parameters replicated.  Per core:

  A: multi-scale dilated conv (24 shifted matmuls, bf16) + folded BN +
     SiLU (ScalarE, conv bias as activation bias)
  A2: bottleneck low = (alpha W_low)^T @ silu, accumulated in PSUM
  B: high matmul + residual (folded in as identity matmul) -> LN1 stats
     (DVE bn_stats) -> normalize on ScalarE (per-partition scale/bias)
     -> PE transpose to hT
  C: GAT projections (Wh + src/dst logit columns); src/dst row vectors
     via stacked-column matmuls; replicated srcb only for Y-heads
  D: attention P^T tiles [m,q] via two variants:
     Z: PSUM = mask (identity matmul) + rank-2 (dst[m]+src[q]) matmul,
        then ScalarE Prelu(0.2) -> Exp   (exact additive mask path)
     Y: exp(leaky(s)) = max(e^s, e^{0.2 s}) with e^s = e^dst[m] * e^src[q]
        rank-1: two 4x-mode tensor_scalar_muls + max + min(maskinf)
        (max/min split between DVE and GpSimd)
     hp^T = [Wh | 1]^T @ P^T accumulated in PSUM over m-chunks
  D-tail: per q-chunk: 4 PE transposes (all heads) -> one reciprocal +
     one broadcast-AP multiply
  E: LN2 stats (DVE) + normalize -> DMA out
"""

import numpy as np
import ml_dtypes
from contextlib import ExitStack

import concourse.bass as bass
import concourse.tile as tile
from concourse import bacc, mybir
from concourse.bass_utils import run_bass_kernel_spmd
from concourse.masks import make_identity

BF = mybir.dt.bfloat16
F32 = mybir.dt.float32
EPS = 1e-5
NEG = -1e9
BIG = 1e30

B, N, H = 8, 1024, 256
S, K, HEADS = 4, 3, 4
D = H // HEADS          # 64
NCH = N // 128          # 8
CCH = H // 128          # 2
BOT = 8

AF = mybir.ActivationFunctionType
OP = mybir.AluOpType

# ---- phase D tile assignment ----
# Z-tiles: PE+ScalarE path; Y-tiles: rank-1 DVE path.
ZSET = {(0, j) for j in range(NCH)} | {(1, j) for j in range(4)}
# Y j-ranges per head (contiguous, for batched max/min)
YRANGE = {1: (4, 8), 2: (0, 8), 3: (0, 8)}

_CACHED = {}


def _build() -> bass.Bass:
    nc = bacc.Bacc("TRN2", target_bir_lowering=False, debug=False,
                   num_devices=B)

    decl = nc.declare_dram_parameter
    xpad_d = decl("xpad", [128, CCH * (N + 16)], BF, isOutput=False)
    wt_d = decl("wt", [128, S * K * CCH * H], BF, isOutput=False)
    bconv_d = decl("bconv", [128, S * CCH], F32, isOutput=False)
    wlow_d = decl("wlow", [128, S * CCH * BOT], BF, isOutput=False)
    whigh_d = decl("whigh", [BOT, H], BF, isOutput=False)
    g_d = decl("gmat", [128, CCH * (H + 2 * HEADS)], BF, isOutput=False)
    maska_d = decl("maskadd", [128, NCH * N], BF, isOutput=False)
    minf_d = decl("maskinf", [128, NCH * N], BF, isOutput=False)
    wsr_d = decl("wsrcrep", [128, HEADS * CCH * 128], BF, isOutput=False)
    wsd_d = decl("wsd", [128, CCH * 2 * 33], BF, isOutput=False)
    xres_d = decl("xres", [128, NCH * H], BF, isOutput=False)
    out_d = decl("out", [128, NCH * H], F32, isOutput=True)

    with tile.TileContext(nc) as tc:
        with ExitStack() as ctx:
            _body(ctx, tc, xpad_d, wt_d, bconv_d, wlow_d, whigh_d, g_d,
                  maska_d, minf_d, wsr_d, wsd_d, xres_d, out_d)
    nc.compile()
    return nc


def _body(ctx, tc, xpad_d, wt_d, bconv_d, wlow_d, whigh_d, g_d,
          maska_d, minf_d, wsr_d, wsd_d, xres_d, out_d):
    nc = tc.nc
    consts = ctx.enter_context(tc.tile_pool(name="consts", bufs=1))
    hnp = ctx.enter_context(tc.tile_pool(name="hnp", bufs=2))
    lrlp = ctx.enter_context(tc.tile_pool(name="lrlp", bufs=2))
    statp = ctx.enter_context(tc.tile_pool(name="stats", bufs=4))
    outp = ctx.enter_context(tc.tile_pool(name="outp", bufs=3))
    ptp = ctx.enter_context(tc.tile_pool(name="ptp", bufs=2))

    # ---------------- inputs into SBUF (all contiguous DMAs) --------------
    xpad = consts.tile([128, CCH, N + 16], BF, tag="xpad")
    nc.sync.dma_start(out=xpad[:], in_=xpad_d.rearrange(
        "p (c n) -> p c n", c=CCH))
    wt_sb = consts.tile([128, S * K * CCH, H], BF, tag="wt")
    wt_r = wt_d.rearrange("p (t h) -> p t h", t=S * K * CCH)
    for i4 in range(S):
        nc.sync.dma_start(out=wt_sb[:, i4 * K * CCH:(i4 + 1) * K * CCH, :],
                          in_=wt_r[:, i4 * K * CCH:(i4 + 1) * K * CCH, :])
    bconv_sb = consts.tile([128, S * CCH], F32, tag="bconv")
    nc.sync.dma_start(out=bconv_sb[:], in_=bconv_d[:])
    wlow_sb = consts.tile([128, S * CCH, BOT], BF, tag="wlow")
    nc.sync.dma_start(out=wlow_sb[:], in_=wlow_d.rearrange(
        "p (t b) -> p t b", t=S * CCH))
    whigh_sb = consts.tile([BOT, H], BF, tag="whigh")
    nc.sync.dma_start(out=whigh_sb[:], in_=whigh_d[:])
    g_sb = consts.tile([128, CCH, H + 2 * HEADS], BF, tag="gmat")
    nc.sync.dma_start(out=g_sb[:], in_=g_d.rearrange(
        "p (c h) -> p c h", c=CCH))
    xres_sb = consts.tile([128, NCH, H], BF, tag="xres")
    nc.sync.dma_start(out=xres_sb[:], in_=xres_d.rearrange(
        "p (n h) -> p n h", n=NCH))
    wsd_sb = consts.tile([128, CCH, 2, 33], BF, tag="wsd")
    nc.sync.dma_start(out=wsd_sb[:], in_=wsd_d.rearrange(
        "p (c s h) -> p c s h", c=CCH, s=2))
    wsr_sb = consts.tile([128, HEADS, CCH, 128], BF, tag="wsr")
    nc.sync.dma_start(out=wsr_sb[:], in_=wsr_d.rearrange(
        "p (h c x) -> p h c x", h=HEADS, c=CCH))
    maska_sb = consts.tile([128, NCH, N], BF, tag="maska")
    nc.sync.dma_start(out=maska_sb[:], in_=maska_d.rearrange(
        "p (j n) -> p j n", j=NCH))
    minf_sb = consts.tile([128, NCH, N], BF, tag="minf")
    nc.sync.dma_start(out=minf_sb[:], in_=minf_d.rearrange(
        "p (j n) -> p j n", j=NCH))

    ident_bf = consts.tile([128, 128], BF, tag="idbf")
    make_identity(nc, ident_bf[:])
    ident_f32 = consts.tile([128, 128], F32, tag="idf32")
    make_identity(nc, ident_f32[:])
    eps_sb = consts.tile([128, 1], F32, tag="eps")
    nc.vector.memset(eps_sb[:], EPS)
    zero_sb = consts.tile([128, 1], F32, tag="zero")
    nc.vector.memset(zero_sb[:], 0.0)

    # persistent intermediates
    fused_sb = consts.tile([128, S, CCH, N], BF, tag="fused")
    lowT_sb = consts.tile([BOT, N], BF, tag="lowT")
    h_all = consts.tile([128, NCH, H], F32, tag="h_all")
    mv1 = consts.tile([128, NCH, 2], F32, tag="mv1")
    rstd1 = consts.tile([128, NCH], F32, tag="rstd1")
    negmr1 = consts.tile([128, NCH], F32, tag="negmr1")
    hT_sb = consts.tile([128, CCH, N], BF, tag="hT")
    wh_all = consts.tile([128, NCH, HEADS * (D + 1)], BF, tag="wh")
    nc.vector.memset(
        wh_all[:].rearrange("p j (h x) -> p j h x", x=D + 1)[:, :, :, D], 1.0)
    sd_sb = consts.tile([128, NCH, 2 * HEADS], F32, tag="sd")
    expdst = consts.tile([128, NCH, HEADS], F32, tag="expdst")
    expdst02 = consts.tile([128, NCH, HEADS], F32, tag="expdst02")
    # src row vectors: srsp[g] holds src_{2g} at partition 0, src_{2g+1} at 32
    srsp = [consts.tile([33, N], BF, tag=f"srsp{g}", name=f"srsp{g}")
            for g in range(2)]
    ones1 = consts.tile([33, 128], BF, tag="ones1")
    nc.vector.memset(ones1[0:1, :], 1.0)
    nc.vector.memset(ones1[32:33, :], 1.0)
    brep_all = consts.tile([128, HEADS, N], BF, tag="brep")
    drep_all = consts.tile([128, HEADS, N], BF, tag="drep")
    t1a = consts.tile([128, NCH, N], BF, tag="t1a")
    t2a = consts.tile([128, NCH, N], BF, tag="t2a")
    hptall = consts.tile([D + 1, HEADS, N], BF, tag="hptall")
    hp_all = consts.tile([128, NCH, H], F32, tag="hp")
    mv2 = consts.tile([128, NCH, 2], F32, tag="mv2")
    rstd2 = consts.tile([128, NCH], F32, tag="rstd2")
    negmr2 = consts.tile([128, NCH], F32, tag="negmr2")

    yheads = sorted({h for h in range(HEADS)
                     for j in range(NCH) if (h, j) not in ZSET})

    # ---------------- phase A: conv + silu ----------------
    ctxA = ExitStack()
    convp = ctxA.enter_context(tc.tile_pool(name="convp", bufs=2, space="PSUM"))
    lowp = ctxA.enter_context(tc.tile_pool(name="lowp", bufs=2, space="PSUM"))
    for cout in range(CCH):
        for i in range(S):
            ps = convp.tile([128, 1024], F32, tag="conv")
            dil = 2 ** i
            for nch in range(2):
                first = True
                for c in range(CCH):
                    for k in range(K):
                        sh = (k - 1) * dil
                        t = (i * K + k) * CCH + c
                        nc.tensor.matmul(
                            ps[:, nch * 512:nch * 512 + 512],
                            lhsT=wt_sb[:, t, cout * 128:(cout + 1) * 128],
                            rhs=xpad[:, c, 8 + sh + nch * 512:
                                     8 + sh + nch * 512 + 512],
                            start=first, stop=(c == CCH - 1 and k == K - 1))
                        first = False
            nc.scalar.activation(
                out=fused_sb[:, i, cout, :], in_=ps[:],
                func=AF.Silu,
                bias=bconv_sb[:, i * CCH + cout:i * CCH + cout + 1], scale=1.0)

    # -------- phase A2: lowT = sum_i (a_i W_low)^T @ silu_i --------
    for nch in range(2):
        lps = lowp.tile([BOT, 512], F32, tag="low")
        first = True
        for i in range(S):
            for c in range(CCH):
                nc.tensor.matmul(
                    lps[:],
                    lhsT=wlow_sb[:, i * CCH + c, :],
                    rhs=fused_sb[:, i, c, nch * 512:nch * 512 + 512],
                    start=first, stop=(i == S - 1 and c == CCH - 1))
                first = False
        nc.vector.tensor_copy(out=lowT_sb[:, nch * 512:nch * 512 + 512],
                              in_=lps[:])
    ctxA.close()

    # ------- phase B: high + residual (identity matmul) + ln1 + hT -------
    ctxB = ExitStack()
    psB = ctxB.enter_context(tc.tile_pool(name="psB", bufs=3, space="PSUM"))
    psTr = ctxB.enter_context(tc.tile_pool(name="psTrB", bufs=2, space="PSUM"))
    for q in range(NCH):
        hps = psB.tile([128, H], F32, tag="high")
        nc.tensor.matmul(hps[:], lhsT=lowT_sb[:, q * 128:(q + 1) * 128],
                         rhs=whigh_sb[:], start=True, stop=False)
        nc.tensor.matmul(hps[:], lhsT=ident_bf[:], rhs=xres_sb[:, q, :],
                         start=False, stop=True)
        st = statp.tile([128, 6], F32, tag="bn1")
        nc.vector.bn_stats(out=st[:], in_=hps[:])
        nc.vector.bn_aggr(out=mv1[:, q, :], in_=st[:])
        nc.vector.tensor_copy(out=h_all[:, q, :], in_=hps[:])

    nc.scalar.activation(out=rstd1[:], in_=mv1[:, :, 1], func=AF.Ln,
                         bias=eps_sb[:], scale=1.0)
    nc.scalar.activation(out=rstd1[:], in_=rstd1[:], func=AF.Exp,
                         bias=zero_sb[:], scale=-0.5)
    nc.vector.scalar_tensor_tensor(
        out=negmr1[:], in0=mv1[:, :, 0], scalar=-1.0, in1=rstd1[:],
        op0=OP.mult, op1=OP.mult)

    for q in range(NCH):
        hn = hnp.tile([128, H], BF, tag="hn")
        nc.scalar.activation(out=hn[:], in_=h_all[:, q, :], func=AF.Identity,
                             bias=negmr1[:, q:q + 1], scale=rstd1[:, q:q + 1])
        tp = psTr.tile([128, CCH, 128], BF, tag="trh")
        for c in range(CCH):
            nc.tensor.transpose(out=tp[:, c, :], in_=hn[:, c * 128:(c + 1) * 128],
                                identity=ident_bf[:])
        nc.vector.tensor_copy(out=hT_sb[:, :, q * 128:(q + 1) * 128], in_=tp[:])
    ctxB.close()

    # ---------------- phase C: GAT projections ----------------
    ctxC = ExitStack()
    psC = ctxC.enter_context(tc.tile_pool(name="psC", bufs=2, space="PSUM"))
    psR = ctxC.enter_context(tc.tile_pool(name="psRC", bufs=2, space="PSUM"))
    for j in range(NCH):
        gps = psC.tile([128, H + 2 * HEADS], F32, tag="gat")
        for c in range(CCH):
            nc.tensor.matmul(gps[:], lhsT=hT_sb[:, c, j * 128:(j + 1) * 128],
                             rhs=g_sb[:, c, :], start=(c == 0),
                             stop=(c == CCH - 1))
        whj = wh_all[:, j, :].rearrange("p (h x) -> p h x", x=D + 1)
        nc.vector.tensor_copy(
            out=whj[:, :, 0:D],
            in_=gps[:, 0:H].rearrange("p (h x) -> p h x", x=D))
        nc.vector.tensor_copy(out=sd_sb[:, j, :], in_=gps[:, H:H + 2 * HEADS])

    # src row vectors, spread to partitions {0, 32} of two tiles
    for g in range(2):
        for half in range(2):
            sl = slice(half * 512, half * 512 + 512)
            srps = psR.tile([33, 512], F32, tag="srps")
            for c in range(CCH):
                nc.tensor.matmul(srps[:], lhsT=wsd_sb[:, c, g, :],
                                 rhs=hT_sb[:, c, sl],
                                 start=(c == 0), stop=(c == CCH - 1))
            nc.vector.tensor_copy(out=srsp[g][:, sl], in_=srps[:])

    # replicated srcb for Y-heads only
    for h in yheads:
        for half in range(2):
            sl = slice(half * 512, half * 512 + 512)
            sps = psR.tile([128, 512], F32, tag="sbc")
            for c in range(CCH):
                nc.tensor.matmul(
                    sps[:], lhsT=wsr_sb[:, h, c, :], rhs=hT_sb[:, c, sl],
                    start=(c == 0), stop=(c == CCH - 1))
            nc.scalar.activation(out=brep_all[:, h, sl], in_=sps[:],
                                 func=AF.Exp, bias=zero_sb[:], scale=1.0)
            nc.scalar.activation(out=drep_all[:, h, sl], in_=sps[:],
                                 func=AF.Exp, bias=zero_sb[:], scale=0.2)

    # per-partition exp(dst), exp(0.2 dst) for Y tiles
    dslice = sd_sb[:].rearrange("p j (s h) -> p s j h", s=2)[:, 1]
    nc.scalar.activation(out=expdst[:], in_=dslice, func=AF.Exp,
                         bias=zero_sb[:], scale=1.0)
    nc.scalar.activation(out=expdst02[:], in_=dslice, func=AF.Exp,
                         bias=zero_sb[:], scale=0.2)
    ctxC.close()

    # ---------------- phase D: attention ----------------
    ctxD = ExitStack()
    attp = ctxD.enter_context(tc.tile_pool(name="attp", bufs=3, space="PSUM"))
    zpsp = ctxD.enter_context(tc.tile_pool(name="zpsp", bufs=2, space="PSUM"))
    tqp = ctxD.enter_context(tc.tile_pool(name="tqp", bufs=1, space="PSUM"))
    for h in range(HEADS):
        pt = ptp.tile([128, NCH, N], BF, tag="pt")
        hp0 = attp.tile([D + 1, 512], F32, tag="hpT")
        hp1 = attp.tile([D + 1, 512], F32, tag="hpT")
        hp_halves = (hp0[:], hp1[:])
        for j in range(NCH):
            if (h, j) in ZSET:
                # --- Z: PSUM = mask + (dst[m] + src[q]); Prelu; Exp ---
                zps = zpsp.tile([128, 1024], F32, tag="zps")
                row = 32 * (h % 2)
                for half in range(2):
                    sl = slice(half * 512, half * 512 + 512)
                    nc.tensor.matmul(
                        zps[:, sl], lhsT=ident_bf[:],
                        rhs=maska_sb[:, j, sl], start=True, stop=False)
                    nc.tensor.matmul(
                        zps[:, sl], lhsT=ones1[row:row + 1, :],
                        rhs=srsp[h // 2][row:row + 1, sl],
                        start=False, stop=True)
                lrl = lrlp.tile([128, N], BF, tag="lrl")
                nc.scalar.activation(
                    out=lrl[:], in_=zps[:], func=AF.Prelu,
                    bias=sd_sb[:, j, HEADS + h:HEADS + h + 1],
                    scale=1.0, alpha=0.2)
                nc.scalar.activation(out=pt[:, j, :], in_=lrl[:], func=AF.Exp,
                                     bias=zero_sb[:], scale=1.0)
            else:
                # --- Y: rank-1 max(e^s, e^0.2s); mask-min batched below ---
                nc.vector.tensor_scalar_mul(
                    out=t1a[:, j, :], in0=brep_all[:, h, :],
                    scalar1=expdst[:, j, h:h + 1])
                nc.vector.tensor_scalar_mul(
                    out=t2a[:, j, :], in0=drep_all[:, h, :],
                    scalar1=expdst02[:, j, h:h + 1])
            if (h, j) in ZSET:
                for half in range(2):
                    nc.tensor.matmul(
                        hp_halves[half],
                        lhsT=wh_all[:, j, h * (D + 1):(h + 1) * (D + 1)],
                        rhs=pt[:, j, half * 512:half * 512 + 512],
                        start=(j == 0),
                        stop=(j == NCH - 1 and h not in YRANGE))
        if h in YRANGE:
            j0, j1 = YRANGE[h]
            nc.vector.tensor_tensor(
                out=t1a[:, j0:j1, :], in0=t1a[:, j0:j1, :],
                in1=t2a[:, j0:j1, :], op=OP.max)
            nc.vector.tensor_tensor(
                out=pt[:, j0:j1, :], in0=t1a[:, j0:j1, :],
                in1=minf_sb[:, j0:j1, :], op=OP.min)
            for j in range(j0, j1):
                for half in range(2):
                    nc.tensor.matmul(
                        hp_halves[half],
                        lhsT=wh_all[:, j, h * (D + 1):(h + 1) * (D + 1)],
                        rhs=pt[:, j, half * 512:half * 512 + 512],
                        start=(j == 0), stop=(j == NCH - 1))
        nc.scalar.activation(out=hptall[:, h, 0:512], in_=hp0[:],
                             func=AF.Identity, bias=zero_sb[0:D + 1],
                             scale=1.0)
        nc.scalar.activation(out=hptall[:, h, 512:N], in_=hp1[:],
                             func=AF.Identity, bias=zero_sb[0:D + 1],
                             scale=1.0)

    # D-tail: per q-chunk transpose all heads, one recip, one bcast mult
    for q in range(NCH):
        tq = tqp.tile([128, HEADS, D + 4], BF, tag="tq4")
        for h in range(HEADS):
            nc.tensor.transpose(out=tq[:, h, 0:D + 1],
                                in_=hptall[:, h, q * 128:(q + 1) * 128],
                                identity=ident_bf[0:D + 1, 0:D + 1])
        rd = statp.tile([128, HEADS], F32, tag="rd")
        nc.vector.reciprocal(out=rd[:], in_=tq[:, :, D])
        nc.vector.tensor_tensor(
            out=hp_all[:, q, :].rearrange("p (h d) -> p h d", d=D),
            in0=tq[:, :, 0:D],
            in1=rd[:].rearrange("p (h o) -> p h o", o=1).broadcast_to(
                [128, HEADS, D]),
            op=OP.mult)
    ctxD.close()

    # ---------------- phase E: ln2 + out ----------------
    for q in range(NCH):
        st = statp.tile([128, 6], F32, tag="bn2")
        nc.vector.bn_stats(out=st[:], in_=hp_all[:, q, :])
        nc.vector.bn_aggr(out=mv2[:, q, :], in_=st[:])
    nc.scalar.activation(out=rstd2[:], in_=mv2[:, :, 1], func=AF.Ln,
                         bias=eps_sb[:], scale=1.0)
    nc.scalar.activation(out=rstd2[:], in_=rstd2[:], func=AF.Exp,
                         bias=zero_sb[:], scale=-0.5)
    nc.vector.scalar_tensor_tensor(
        out=negmr2[:], in0=mv2[:, :, 0], scalar=-1.0, in1=rstd2[:],
        op0=OP.mult, op1=OP.mult)
    for q in range(NCH):
        ot = outp.tile([128, H], F32, tag="out")
        nc.scalar.activation(out=ot[:], in_=hp_all[:, q, :],
                             func=AF.Identity, bias=negmr2[:, q:q + 1],
                             scale=rstd2[:, q:q + 1])
        nc.sync.dma_start(out=out_d[:, q * H:(q + 1) * H], in_=ot[:])


def _prep(inputs):
    """Host-side parameter folding. Returns per-core input maps."""
    bf16 = ml_dtypes.bfloat16
    f = lambda a: np.ascontiguousarray(np.asarray(a, np.float32))

    x = f(inputs["x"])
    adj = np.asarray(inputs["adj"])
    conv_w = f(inputs["conv_w"]); conv_b = f(inputs["conv_b"])
    bn_g = f(inputs["bn_g"]); bn_b = f(inputs["bn_b"])
    fw = f(inputs["fusion_weight"])
    W_low = f(inputs["W_low"]); b_low = f(inputs["b_low"])
    W_high = f(inputs["W_high"]); b_high = f(inputs["b_high"])
    ln1_g = f(inputs["ln1_g"]); ln1_b = f(inputs["ln1_b"])
    gat_W = f(inputs["gat_W"])
    a_src = f(inputs["a_src"]); a_dst = f(inputs["a_dst"])
    ln2_g = f(inputs["ln2_g"]); ln2_b = f(inputs["ln2_b"])

    trivial = dict(
        b_low=np.allclose(b_low, 0), b_high=np.allclose(b_high, 0),
        ln1=np.allclose(ln1_g, 1) and np.allclose(ln1_b, 0),
        ln2=np.allclose(ln2_g, 1) and np.allclose(ln2_b, 0))
    if not all(trivial.values()):
        raise NotImplementedError(f"non-trivial affine params: {trivial}")

    alpha = np.exp(fw - fw.max()); alpha /= alpha.sum()
    gprime = bn_g / np.float32(np.sqrt(1.0 + EPS))          # [S,H]
    bconv = conv_b * gprime + bn_b                           # [S,H]
    Wt = np.transpose(conv_w, (0, 3, 2, 1)) * gprime[:, None, None, :]
    Wt = Wt.reshape(S * K * CCH, 128, H).transpose(1, 0, 2)  # [128,24,H]
    bconv_t = bconv.reshape(S, CCH, 128).transpose(2, 0, 1).reshape(128, S * CCH)

    WlowA = (alpha[:, None, None] * W_low[None]).reshape(S * CCH, 128, BOT)
    WlowA = WlowA.transpose(1, 0, 2)                         # [128,S*CCH,BOT]

    G = np.zeros((H, H + 2 * HEADS), np.float32)
    wsrc = np.zeros((HEADS, H), np.float32)
    wdst = np.zeros((HEADS, H), np.float32)
    for h in range(HEADS):
        G[:, h * D:(h + 1) * D] = gat_W[h]
        wsrc[h] = gat_W[h] @ a_src[h]
        wdst[h] = gat_W[h] @ a_dst[h]
        G[:, H + h] = wsrc[h]
        G[:, H + HEADS + h] = wdst[h]
    Gr = G.reshape(CCH, 128, H + 2 * HEADS).transpose(1, 0, 2)

    maskadd = np.where(adj.T > 0, np.float32(0.0), np.float32(NEG))
    maskadd = maskadd.reshape(NCH, 128, N).transpose(1, 0, 2)
    maskinf = np.where(adj.T > 0, np.float32(BIG), np.float32(0.0))
    maskinf = maskinf.reshape(NCH, 128, N).transpose(1, 0, 2)

    # wsrcrep[p, h, c, :] = wsrc[h][c*128+p] replicated over last axis
    wsrep = np.repeat(wsrc.reshape(HEADS, CCH, 128, 1), 128, axis=3)
    wsrep = wsrep.transpose(2, 0, 1, 3)                      # [128,H,C,128]

    # wsd[p, c, g, :]: 33 columns; col 0 = wsrc[2g], col 32 = wsrc[2g+1]
    wsd = np.zeros((128, CCH, 2, 33), np.float32)
    wsr_c = wsrc.reshape(HEADS, CCH, 128)                    # [H,C,128]
    for g in range(2):
        wsd[:, :, g, 0] = wsr_c[2 * g].T
        wsd[:, :, g, 32] = wsr_c[2 * g + 1].T

    cc = np.ascontiguousarray
    shared = {
        "wt": cc(Wt.reshape(128, -1)).astype(bf16),
        "bconv": cc(bconv_t),
        "wlow": cc(WlowA.reshape(128, -1)).astype(bf16),
        "whigh": cc(W_high).astype(bf16),
        "gmat": cc(Gr.reshape(128, -1)).astype(bf16),
        "maskadd": cc(maskadd.reshape(128, -1)).astype(bf16),
        "maskinf": cc(maskinf.reshape(128, -1)).astype(bf16),
        "wsrcrep": cc(wsrep.reshape(128, -1)).astype(bf16),
        "wsd": cc(wsd.reshape(128, -1)).astype(bf16),
    }
    in_maps = []
    for b in range(B):
        xt = x[b].T                                          # [H, N]
        xp = np.zeros((128, CCH, N + 16), np.float32)
        xp[:, :, 8:8 + N] = xt.reshape(CCH, 128, N).transpose(1, 0, 2)
        xr = x[b].reshape(NCH, 128, H).transpose(1, 0, 2)
        m = dict(shared)
        m["xpad"] = cc(xp.reshape(128, -1)).astype(bf16)
        m["xres"] = cc(xr.reshape(128, -1)).astype(bf16)
        in_maps.append(m)
    return in_maps, trivial


def kernel(**inputs) -> np.ndarray:
    in_maps, trivial = _prep(inputs)
    key = "k"
    if key not in _CACHED:
        _CACHED[key] = _build()
    nc = _CACHED[key]
    res = run_bass_kernel_spmd(nc, in_maps, list(range(B)))
    out = np.stack(
        [res.results[i]["out"].reshape(128, NCH, H).transpose(1, 0, 2)
         .reshape(N, H) for i in range(B)], axis=0)
    return out.astype(np.float32)


if __name__ == "__main__":
    import reference
    inputs = {k: np.asarray(v) for k, v in reference.setup_inputs().items()}
    got = kernel(**inputs)
    print("kernel output", got.shape, got.dtype)


# revision 37
# speedup vs baseline: 1.0980x; 1.0980x over previous
"""Trainium2 Bass kernel for nn_LocationAwareMSAGAT_Net.

Data-parallel over batch B=8 across 8 NeuronCores (one batch element per
core); parameters replicated.  Per core:

  A: multi-scale dilated conv (24 shifted matmuls, bf16) + folded BN +
     SiLU (ScalarE, conv bias as activation bias)
  A2: bottleneck low = (alpha W_low)^T @ silu, accumulated in PSUM
  B: high matmul + residual (folded in as identity matmul) -> LN1 stats
     (DVE bn_stats) -> normalize on ScalarE (per-partition scale/bias)
     -> PE transpose to hT
  C: GAT projections (Wh + src/dst logit columns); src/dst row vectors
     via stacked-column matmuls; replicated srcb only for Y-heads
  D: attention P^T tiles [m,q] via two variants:
     Z: PSUM = mask (identity matmul) + rank-2 (dst[m]+src[q]) matmul,
        then ScalarE Prelu(0.2) -> Exp   (exact additive mask path)
     Y: exp(leaky(s)) = max(e^s, e^{0.2 s}) with e^s = e^dst[m] * e^src[q]
        rank-1: two 4x-mode tensor_scalar_muls + max + min(maskinf)
        (max/min split between DVE and GpSimd)
     hp^T = [Wh | 1]^T @ P^T accumulated in PSUM over m-chunks
  D-tail: per q-chunk: 4 PE transposes (all heads) -> one reciprocal +
     one broadcast-AP multiply
  E: LN2 stats (DVE) + normalize -> DMA out
"""

import numpy as np
import ml_dtypes
from contextlib import ExitStack

import concourse.bass as bass
import concourse.tile as tile
from concourse import bacc, mybir
from concourse.bass_utils import run_bass_kernel_spmd
from concourse.masks import make_identity

BF = mybir.dt.bfloat16
F32 = mybir.dt.float32
EPS = 1e-5
NEG = -1e9
BIG = 1e30

B, N, H = 8, 1024, 256
S, K, HEADS = 4, 3, 4
D = H // HEADS          # 64
NCH = N // 128          # 8
CCH = H // 128          # 2
BOT = 8

AF = mybir.ActivationFunctionType
OP = mybir.AluOpType

# ---- phase D tile assignment ----
# Z-tiles: PE+ScalarE path; Y-tiles: rank-1 DVE path.
ZSET = {(h, j) for h in range(HEADS) for j in ((h) % 8, (h + 2) % 8, (h + 4) % 8)}

_CACHED = {}


def _build() -> bass.Bass:
    nc = bacc.Bacc("TRN2", target_bir_lowering=False, debug=False,
                   num_devices=B)

    decl = nc.declare_dram_parameter
    xpad_d = decl("xpad", [128, CCH * (N + 16)], BF, isOutput=False)
    wt_d = decl("wt", [128, S * K * CCH * H], BF, isOutput=False)
    bconv_d = decl("bconv", [128, S * CCH], F32, isOutput=False)
    wlow_d = decl("wlow", [128, S * CCH * BOT], BF, isOutput=False)
    whigh_d = decl("whigh", [BOT, H], BF, isOutput=False)
    g_d = decl("gmat", [128, CCH * (H + 2 * HEADS)], BF, isOutput=False)
    maska_d = decl("maskadd", [128, NCH * N], BF, isOutput=False)
    minf_d = decl("maskinf", [128, NCH * N], BF, isOutput=False)
    wsr_d = decl("wsrcrep", [128, HEADS * CCH * 128], BF, isOutput=False)
    wsd_d = decl("wsd", [128, CCH * 2 * 33], BF, isOutput=False)
    xres_d = decl("xres", [128, NCH * H], BF, isOutput=False)
    out_d = decl("out", [128, NCH * H], F32, isOutput=True)

    with tile.TileContext(nc) as tc:
        with ExitStack() as ctx:
            _body(ctx, tc, xpad_d, wt_d, bconv_d, wlow_d, whigh_d, g_d,
                  maska_d, minf_d, wsr_d, wsd_d, xres_d, out_d)
    nc.compile()
    return nc


def _body(ctx, tc, xpad_d, wt_d, bconv_d, wlow_d, whigh_d, g_d,
          maska_d, minf_d, wsr_d, wsd_d, xres_d, out_d):
    nc = tc.nc
    consts = ctx.enter_context(tc.tile_pool(name="consts", bufs=1))
    hnp = ctx.enter_context(tc.tile_pool(name="hnp", bufs=2))
    lrlp = ctx.enter_context(tc.tile_pool(name="lrlp", bufs=2))
    typ = ctx.enter_context(tc.tile_pool(name="typ", bufs=2))
    statp = ctx.enter_context(tc.tile_pool(name="stats", bufs=4))
    outp = ctx.enter_context(tc.tile_pool(name="outp", bufs=3))
    ptp = ctx.enter_context(tc.tile_pool(name="ptp", bufs=2))

    # ---------------- inputs into SBUF (all contiguous DMAs) --------------
    xpad = consts.tile([128, CCH, N + 16], BF, tag="xpad")
    nc.sync.dma_start(out=xpad[:], in_=xpad_d.rearrange(
        "p (c n) -> p c n", c=CCH))
    wt_sb = consts.tile([128, S * K * CCH, H], BF, tag="wt")
    wt_r = wt_d.rearrange("p (t h) -> p t h", t=S * K * CCH)
    for i4 in range(S):
        nc.sync.dma_start(out=wt_sb[:, i4 * K * CCH:(i4 + 1) * K * CCH, :],
                          in_=wt_r[:, i4 * K * CCH:(i4 + 1) * K * CCH, :])
    bconv_sb = consts.tile([128, S * CCH], F32, tag="bconv")
    nc.sync.dma_start(out=bconv_sb[:], in_=bconv_d[:])
    wlow_sb = consts.tile([128, S * CCH, BOT], BF, tag="wlow")
    nc.sync.dma_start(out=wlow_sb[:], in_=wlow_d.rearrange(
        "p (t b) -> p t b", t=S * CCH))
    whigh_sb = consts.tile([BOT, H], BF, tag="whigh")
    nc.sync.dma_start(out=whigh_sb[:], in_=whigh_d[:])
    g_sb = consts.tile([128, CCH, H + 2 * HEADS], BF, tag="gmat")
    nc.sync.dma_start(out=g_sb[:], in_=g_d.rearrange(
        "p (c h) -> p c h", c=CCH))
    xres_sb = consts.tile([128, NCH, H], BF, tag="xres")
    nc.sync.dma_start(out=xres_sb[:], in_=xres_d.rearrange(
        "p (n h) -> p n h", n=NCH))
    wsd_sb = consts.tile([128, CCH, 2, 33], BF, tag="wsd")
    nc.sync.dma_start(out=wsd_sb[:], in_=wsd_d.rearrange(
        "p (c s h) -> p c s h", c=CCH, s=2))
    wsr_sb = consts.tile([128, HEADS, CCH, 128], BF, tag="wsr")
    nc.sync.dma_start(out=wsr_sb[:], in_=wsr_d.rearrange(
        "p (h c x) -> p h c x", h=HEADS, c=CCH))
    maska_sb = consts.tile([128, NCH, N], BF, tag="maska")
    nc.sync.dma_start(out=maska_sb[:], in_=maska_d.rearrange(
        "p (j n) -> p j n", j=NCH))
    minf_sb = consts.tile([128, NCH, N], BF, tag="minf")
    nc.sync.dma_start(out=minf_sb[:], in_=minf_d.rearrange(
        "p (j n) -> p j n", j=NCH))

    ident_bf = consts.tile([128, 128], BF, tag="idbf")
    make_identity(nc, ident_bf[:])
    ident_f32 = consts.tile([128, 128], F32, tag="idf32")
    make_identity(nc, ident_f32[:])
    eps_sb = consts.tile([128, 1], F32, tag="eps")
    nc.vector.memset(eps_sb[:], EPS)
    zero_sb = consts.tile([128, 1], F32, tag="zero")
    nc.vector.memset(zero_sb[:], 0.0)

    # persistent intermediates
    fused_sb = consts.tile([128, S, CCH, N], BF, tag="fused")
    lowT_sb = consts.tile([BOT, N], BF, tag="lowT")
    h_all = consts.tile([128, NCH, H], F32, tag="h_all")
    mv1 = consts.tile([128, NCH, 2], F32, tag="mv1")
    rstd1 = consts.tile([128, NCH], F32, tag="rstd1")
    negmr1 = consts.tile([128, NCH], F32, tag="negmr1")
    hT_sb = consts.tile([128, CCH, N], BF, tag="hT")
    wh_all = consts.tile([128, NCH, HEADS * (D + 1)], BF, tag="wh")
    nc.vector.memset(
        wh_all[:].rearrange("p j (h x) -> p j h x", x=D + 1)[:, :, :, D], 1.0)
    sd_sb = consts.tile([128, NCH, 2 * HEADS], F32, tag="sd")
    expdst = consts.tile([128, NCH, HEADS], F32, tag="expdst")
    expdst02 = consts.tile([128, NCH, HEADS], F32, tag="expdst02")
    # src row vectors: srsp[g] holds src_{2g} at partition 0, src_{2g+1} at 32
    srsp = [consts.tile([33, N], BF, tag=f"srsp{g}", name=f"srsp{g}")
            for g in range(2)]
    ones1 = consts.tile([33, 128], BF, tag="ones1")
    nc.vector.memset(ones1[0:1, :], 1.0)
    nc.vector.memset(ones1[32:33, :], 1.0)
    brep_all = consts.tile([128, HEADS, N], BF, tag="brep")
    drep_all = consts.tile([128, HEADS, N], BF, tag="drep")
    hptall = consts.tile([D + 1, HEADS, N], BF, tag="hptall")
    hp_all = consts.tile([128, NCH, H], F32, tag="hp")
    mv2 = consts.tile([128, NCH, 2], F32, tag="mv2")
    rstd2 = consts.tile([128, NCH], F32, tag="rstd2")
    negmr2 = consts.tile([128, NCH], F32, tag="negmr2")

    yheads = sorted({h for h in range(HEADS)
                     for j in range(NCH) if (h, j) not in ZSET})

    # ---------------- phase A: conv + silu ----------------
    ctxA = ExitStack()
    convp = ctxA.enter_context(tc.tile_pool(name="convp", bufs=2, space="PSUM"))
    lowp = ctxA.enter_context(tc.tile_pool(name="lowp", bufs=2, space="PSUM"))
    for cout in range(CCH):
        for i in range(S):
            ps = convp.tile([128, 1024], F32, tag="conv")
            dil = 2 ** i
            for nch in range(2):
                first = True
                for c in range(CCH):
                    for k in range(K):
                        sh = (k - 1) * dil
                        t = (i * K + k) * CCH + c
                        nc.tensor.matmul(
                            ps[:, nch * 512:nch * 512 + 512],
                            lhsT=wt_sb[:, t, cout * 128:(cout + 1) * 128],
                            rhs=xpad[:, c, 8 + sh + nch * 512:
                                     8 + sh + nch * 512 + 512],
                            start=first, stop=(c == CCH - 1 and k == K - 1))
                        first = False
            nc.scalar.activation(
                out=fused_sb[:, i, cout, :], in_=ps[:],
                func=AF.Silu,
                bias=bconv_sb[:, i * CCH + cout:i * CCH + cout + 1], scale=1.0)

    # -------- phase A2: lowT = sum_i (a_i W_low)^T @ silu_i --------
    for nch in range(2):
        lps = lowp.tile([BOT, 512], F32, tag="low")
        first = True
        for i in range(S):
            for c in range(CCH):
                nc.tensor.matmul(
                    lps[:],
                    lhsT=wlow_sb[:, i * CCH + c, :],
                    rhs=fused_sb[:, i, c, nch * 512:nch * 512 + 512],
                    start=first, stop=(i == S - 1 and c == CCH - 1))
                first = False
        nc.vector.tensor_copy(out=lowT_sb[:, nch * 512:nch * 512 + 512],
                              in_=lps[:])
    ctxA.close()

    # ------- phase B: high + residual (identity matmul) + ln1 + hT -------
    ctxB = ExitStack()
    psB = ctxB.enter_context(tc.tile_pool(name="psB", bufs=3, space="PSUM"))
    psTr = ctxB.enter_context(tc.tile_pool(name="psTrB", bufs=2, space="PSUM"))
    for q in range(NCH):
        hps = psB.tile([128, H], F32, tag="high")
        nc.tensor.matmul(hps[:], lhsT=lowT_sb[:, q * 128:(q + 1) * 128],
                         rhs=whigh_sb[:], start=True, stop=False)
        nc.tensor.matmul(hps[:], lhsT=ident_bf[:], rhs=xres_sb[:, q, :],
                         start=False, stop=True)
        st = statp.tile([128, 6], F32, tag="bn1")
        nc.vector.bn_stats(out=st[:], in_=hps[:])
        nc.vector.bn_aggr(out=mv1[:, q, :], in_=st[:])
        nc.vector.tensor_copy(out=h_all[:, q, :], in_=hps[:])

    nc.scalar.activation(out=rstd1[:], in_=mv1[:, :, 1], func=AF.Ln,
                         bias=eps_sb[:], scale=1.0)
    nc.scalar.activation(out=rstd1[:], in_=rstd1[:], func=AF.Exp,
                         bias=zero_sb[:], scale=-0.5)
    nc.vector.scalar_tensor_tensor(
        out=negmr1[:], in0=mv1[:, :, 0], scalar=-1.0, in1=rstd1[:],
        op0=OP.mult, op1=OP.mult)

    for q in range(NCH):
        hn = hnp.tile([128, H], BF, tag="hn")
        nc.scalar.activation(out=hn[:], in_=h_all[:, q, :], func=AF.Identity,
                             bias=negmr1[:, q:q + 1], scale=rstd1[:, q:q + 1])
        tp = psTr.tile([128, CCH, 128], BF, tag="trh")
        for c in range(CCH):
            nc.tensor.transpose(out=tp[:, c, :], in_=hn[:, c * 128:(c + 1) * 128],
                                identity=ident_bf[:])
        nc.vector.tensor_copy(out=hT_sb[:, :, q * 128:(q + 1) * 128], in_=tp[:])
    ctxB.close()

    # ---------------- phase C: GAT projections ----------------
    ctxC = ExitStack()
    psC = ctxC.enter_context(tc.tile_pool(name="psC", bufs=2, space="PSUM"))
    psR = ctxC.enter_context(tc.tile_pool(name="psRC", bufs=2, space="PSUM"))
    for j in range(NCH):
        gps = psC.tile([128, H + 2 * HEADS], F32, tag="gat")
        for c in range(CCH):
            nc.tensor.matmul(gps[:], lhsT=hT_sb[:, c, j * 128:(j + 1) * 128],
                             rhs=g_sb[:, c, :], start=(c == 0),
                             stop=(c == CCH - 1))
        whj = wh_all[:, j, :].rearrange("p (h x) -> p h x", x=D + 1)
        nc.vector.tensor_copy(
            out=whj[:, :, 0:D],
            in_=gps[:, 0:H].rearrange("p (h x) -> p h x", x=D))
        nc.vector.tensor_copy(out=sd_sb[:, j, :], in_=gps[:, H:H + 2 * HEADS])

    # src row vectors, spread to partitions {0, 32} of two tiles
    for g in range(2):
        for half in range(2):
            sl = slice(half * 512, half * 512 + 512)
            srps = psR.tile([33, 512], F32, tag="srps")
            for c in range(CCH):
                nc.tensor.matmul(srps[:], lhsT=wsd_sb[:, c, g, :],
                                 rhs=hT_sb[:, c, sl],
                                 start=(c == 0), stop=(c == CCH - 1))
            nc.vector.tensor_copy(out=srsp[g][:, sl], in_=srps[:])

    # replicated srcb for Y-heads only
    for h in yheads:
        for half in range(2):
            sl = slice(half * 512, half * 512 + 512)
            sps = psR.tile([128, 512], F32, tag="sbc")
            for c in range(CCH):
                nc.tensor.matmul(
                    sps[:], lhsT=wsr_sb[:, h, c, :], rhs=hT_sb[:, c, sl],
                    start=(c == 0), stop=(c == CCH - 1))
            nc.scalar.activation(out=brep_all[:, h, sl], in_=sps[:],
                                 func=AF.Exp, bias=zero_sb[:], scale=1.0)
            nc.scalar.activation(out=drep_all[:, h, sl], in_=sps[:],
                                 func=AF.Exp, bias=zero_sb[:], scale=0.2)

    # per-partition exp(dst), exp(0.2 dst) for Y tiles
    dslice = sd_sb[:].rearrange("p j (s h) -> p s j h", s=2)[:, 1]
    nc.scalar.activation(out=expdst[:], in_=dslice, func=AF.Exp,
                         bias=zero_sb[:], scale=1.0)
    nc.scalar.activation(out=expdst02[:], in_=dslice, func=AF.Exp,
                         bias=zero_sb[:], scale=0.2)
    ctxC.close()

    # ---------------- phase D: attention ----------------
    ctxD = ExitStack()
    attp = ctxD.enter_context(tc.tile_pool(name="attp", bufs=3, space="PSUM"))
    zpsp = ctxD.enter_context(tc.tile_pool(name="zpsp", bufs=2, space="PSUM"))
    tqp = ctxD.enter_context(tc.tile_pool(name="tqp", bufs=1, space="PSUM"))
    for h in range(HEADS):
        pt = ptp.tile([128, NCH, N], BF, tag="pt")
        hp0 = attp.tile([D + 1, 512], F32, tag="hpT")
        hp1 = attp.tile([D + 1, 512], F32, tag="hpT")
        hp_halves = (hp0[:], hp1[:])
        for j in range(NCH):
            if (h, j) in ZSET:
                # --- Z: PSUM = mask + (dst[m] + src[q]); Prelu; Exp ---
                zps = zpsp.tile([128, 1024], F32, tag="zps")
                row = 32 * (h % 2)
                for half in range(2):
                    sl = slice(half * 512, half * 512 + 512)
                    nc.tensor.matmul(
                        zps[:, sl], lhsT=ident_bf[:],
                        rhs=maska_sb[:, j, sl], start=True, stop=False)
                    nc.tensor.matmul(
                        zps[:, sl], lhsT=ones1[row:row + 1, :],
                        rhs=srsp[h // 2][row:row + 1, sl],
                        start=False, stop=True)
                lrl = lrlp.tile([128, N], BF, tag="lrl")
                nc.scalar.activation(
                    out=lrl[:], in_=zps[:], func=AF.Prelu,
                    bias=sd_sb[:, j, HEADS + h:HEADS + h + 1],
                    scale=1.0, alpha=0.2)
                nc.scalar.activation(out=pt[:, j, :], in_=lrl[:], func=AF.Exp,
                                     bias=zero_sb[:], scale=1.0)
            else:
                # --- Y: rank-1 max(e^s, e^0.2s), min-mask ---
                t1 = typ.tile([128, N], BF, tag="t1")
                t2 = typ.tile([128, N], BF, tag="t2")
                t3 = typ.tile([128, N], BF, tag="t3")
                nc.vector.tensor_scalar_mul(
                    out=t1[:], in0=brep_all[:, h, :],
                    scalar1=expdst[:, j, h:h + 1])
                nc.vector.tensor_scalar_mul(
                    out=t2[:], in0=drep_all[:, h, :],
                    scalar1=expdst02[:, j, h:h + 1])
                nc.vector.tensor_tensor(out=t3[:], in0=t1[:], in1=t2[:],
                                        op=OP.max)
                nc.vector.tensor_tensor(out=pt[:, j, :], in0=t3[:],
                                        in1=minf_sb[:, j, :], op=OP.min)
            for half in range(2):
                nc.tensor.matmul(
                    hp_halves[half],
                    lhsT=wh_all[:, j, h * (D + 1):(h + 1) * (D + 1)],
                    rhs=pt[:, j, half * 512:half * 512 + 512],
                    start=(j == 0), stop=(j == NCH - 1))
        nc.scalar.activation(out=hptall[:, h, 0:512], in_=hp0[:],
                             func=AF.Identity, bias=zero_sb[0:D + 1],
                             scale=1.0)
        nc.scalar.activation(out=hptall[:, h, 512:N], in_=hp1[:],
                             func=AF.Identity, bias=zero_sb[0:D + 1],
                             scale=1.0)

    # D-tail: per q-chunk transpose all heads, one recip, one bcast mult
    for q in range(NCH):
        tq = tqp.tile([128, HEADS, D + 4], BF, tag="tq4")
        for h in range(HEADS):
            nc.tensor.transpose(out=tq[:, h, 0:D + 1],
                                in_=hptall[:, h, q * 128:(q + 1) * 128],
                                identity=ident_bf[0:D + 1, 0:D + 1])
        rd = statp.tile([128, HEADS], F32, tag="rd")
        nc.vector.reciprocal(out=rd[:], in_=tq[:, :, D])
        nc.vector.tensor_tensor(
            out=hp_all[:, q, :].rearrange("p (h d) -> p h d", d=D),
            in0=tq[:, :, 0:D],
            in1=rd[:].rearrange("p (h o) -> p h o", o=1).broadcast_to(
                [128, HEADS, D]),
            op=OP.mult)
    ctxD.close()

    # ---------------- phase E: ln2 + out ----------------
    for q in range(NCH):
        st = statp.tile([128, 6], F32, tag="bn2")
        nc.vector.bn_stats(out=st[:], in_=hp_all[:, q, :])
        nc.vector.bn_aggr(out=mv2[:, q, :], in_=st[:])
    nc.scalar.activation(out=rstd2[:], in_=mv2[:, :, 1], func=AF.Ln,
                         bias=eps_sb[:], scale=1.0)
    nc.scalar.activation(out=rstd2[:], in_=rstd2[:], func=AF.Exp,
                         bias=zero_sb[:], scale=-0.5)
    nc.vector.scalar_tensor_tensor(
        out=negmr2[:], in0=mv2[:, :, 0], scalar=-1.0, in1=rstd2[:],
        op0=OP.mult, op1=OP.mult)
    for q in range(NCH):
        ot = outp.tile([128, H], F32, tag="out")
        nc.scalar.activation(out=ot[:], in_=hp_all[:, q, :],
                             func=AF.Identity, bias=negmr2[:, q:q + 1],
                             scale=rstd2[:, q:q + 1])
        nc.sync.dma_start(out=out_d[:, q * H:(q + 1) * H], in_=ot[:])


def _prep(inputs):
    """Host-side parameter folding. Returns per-core input maps."""
    bf16 = ml_dtypes.bfloat16
    f = lambda a: np.ascontiguousarray(np.asarray(a, np.float32))

    x = f(inputs["x"])
    adj = np.asarray(inputs["adj"])
    conv_w = f(inputs["conv_w"]); conv_b = f(inputs["conv_b"])
    bn_g = f(inputs["bn_g"]); bn_b = f(inputs["bn_b"])
    fw = f(inputs["fusion_weight"])
    W_low = f(inputs["W_low"]); b_low = f(inputs["b_low"])
    W_high = f(inputs["W_high"]); b_high = f(inputs["b_high"])
    ln1_g = f(inputs["ln1_g"]); ln1_b = f(inputs["ln1_b"])
    gat_W = f(inputs["gat_W"])
    a_src = f(inputs["a_src"]); a_dst = f(inputs["a_dst"])
    ln2_g = f(inputs["ln2_g"]); ln2_b = f(inputs["ln2_b"])

    trivial = dict(
        b_low=np.allclose(b_low, 0), b_high=np.allclose(b_high, 0),
        ln1=np.allclose(ln1_g, 1) and np.allclose(ln1_b, 0),
        ln2=np.allclose(ln2_g, 1) and np.allclose(ln2_b, 0))
    if not all(trivial.values()):
        raise NotImplementedError(f"non-trivial affine params: {trivial}")

    alpha = np.exp(fw - fw.max()); alpha /= alpha.sum()
    gprime = bn_g / np.float32(np.sqrt(1.0 + EPS))          # [S,H]
    bconv = conv_b * gprime + bn_b                           # [S,H]
    Wt = np.transpose(conv_w, (0, 3, 2, 1)) * gprime[:, None, None, :]
    Wt = Wt.reshape(S * K * CCH, 128, H).transpose(1, 0, 2)  # [128,24,H]
    bconv_t = bconv.reshape(S, CCH, 128).transpose(2, 0, 1).reshape(128, S * CCH)

    WlowA = (alpha[:, None, None] * W_low[None]).reshape(S * CCH, 128, BOT)
    WlowA = WlowA.transpose(1, 0, 2)                         # [128,S*CCH,BOT]

    G = np.zeros((H, H + 2 * HEADS), np.float32)
    wsrc = np.zeros((HEADS, H), np.float32)
    wdst = np.zeros((HEADS, H), np.float32)
    for h in range(HEADS):
        G[:, h * D:(h + 1) * D] = gat_W[h]
        wsrc[h] = gat_W[h] @ a_src[h]
        wdst[h] = gat_W[h] @ a_dst[h]
        G[:, H + h] = wsrc[h]
        G[:, H + HEADS + h] = wdst[h]
    Gr = G.reshape(CCH, 128, H + 2 * HEADS).transpose(1, 0, 2)

    maskadd = np.where(adj.T > 0, np.float32(0.0), np.float32(NEG))
    maskadd = maskadd.reshape(NCH, 128, N).transpose(1, 0, 2)
    maskinf = np.where(adj.T > 0, np.float32(BIG), np.float32(0.0))
    maskinf = maskinf.reshape(NCH, 128, N).transpose(1, 0, 2)

    # wsrcrep[p, h, c, :] = wsrc[h][c*128+p] replicated over last axis
    wsrep = np.repeat(wsrc.reshape(HEADS, CCH, 128, 1), 128, axis=3)
    wsrep = wsrep.transpose(2, 0, 1, 3)                      # [128,H,C,128]

    # wsd[p, c, g, :]: 33 columns; col 0 = wsrc[2g], col 32 = wsrc[2g+1]
    wsd = np.zeros((128, CCH, 2, 33), np.float32)
    wsr_c = wsrc.reshape(HEADS, CCH, 128)                    # [H,C,128]
    for g in range(2):
        wsd[:, :, g, 0] = wsr_c[2 * g].T
        wsd[:, :, g, 32] = wsr_c[2 * g + 1].T

    cc = np.ascontiguousarray
    shared = {
        "wt": cc(Wt.reshape(128, -1)).astype(bf16),
        "bconv": cc(bconv_t),
        "wlow": cc(WlowA.reshape(128, -1)).astype(bf16),
        "whigh": cc(W_high).astype(bf16),
        "gmat": cc(Gr.reshape(128, -1)).astype(bf16),
        "maskadd": cc(maskadd.reshape(128, -1)).astype(bf16),
        "maskinf": cc(maskinf.reshape(128, -1)).astype(bf16),
        "wsrcrep": cc(wsrep.reshape(128, -1)).astype(bf16),
        "wsd": cc(wsd.reshape(128, -1)).astype(bf16),
    }
    in_maps = []
    for b in range(B):
        xt = x[b].T                                          # [H, N]
        xp = np.zeros((128, CCH, N + 16), np.float32)
        xp[:, :, 8:8 + N] = xt.reshape(CCH, 128, N).transpose(1, 0, 2)
        xr = x[b].reshape(NCH, 128, H).transpose(1, 0, 2)
        m = dict(shared)
        m["xpad"] = cc(xp.reshape(128, -1)).astype(bf16)
        m["xres"] = cc(xr.reshape(128, -1)).astype(bf16)
        in_maps.append(m)
    return in_maps, trivial


def kernel(**inputs) -> np.ndarray:
    in_maps, trivial = _prep(inputs)
    key = "k"
    if key not in _CACHED:
        _CACHED[key] = _build()
    nc = _CACHED[key]
    res = run_bass_kernel_spmd(nc, in_maps, list(range(B)))
    out = np.stack(
        [res.results[i]["out"].reshape(128, NCH, H).transpose(1, 0, 2)
         .reshape(N, H) for i in range(B)], axis=0)
    return out.astype(np.float32)


if __name__ == "__main__":
    import reference
    inputs = {k: np.asarray(v) for k, v in reference.setup_inputs().items()}
    got = kernel(**inputs)
    print("kernel output", got.shape, got.dtype)
